# revision 6
# baseline (speedup 1.0000x reference)
"""Trainium2 Bass kernel for nn_DenoisingTransformer (linear attention transformer block).

Computation (see reference):
  q,k,v = x@Wq, x@Wk, x@Wv  (split into 16 heads of 64)
  q,k = rope(q), rope(k)    (interleaved-pair rope, absolute positions)
  q,k = relu(q), relu(k)
  vk[b,h,e,d] = sum_s v_pad[b,h,s,e] * k[b,h,s,d]   (v padded with ones col, e=65)
  num[b,h,l,e] = sum_d vk[e,d] q[l,d]
  attn = num[..., :64] / (num[..., 64:] + eps)
  out = attn @ Wo

Sharding: 8 cores = (batch 4) x (sequence halves 2). Each core computes its
2048 rows end-to-end; the tiny vk state ([h,65,64] per batch) is all-reduced
between the two cores sharing a batch (replica groups [0,1],[2,3],[4,5],[6,7]).

Design notes (v4):
 - k/v/q projections run in fp8(e4m3) DoubleRow mode (0.5 cycles/row, 2x bf16)
   with a 3-pass residual scheme: x8@W8 + x8@Wr8 + xr8@W8.  W is scaled by 32
   host-side so its residual escapes the e4m3 denormal floor; the 1/32 unscale
   rides the PSUM->SBUF copy.  Measured accuracy: ~2x BETTER than bf16.
 - x ships as uint16 pairs (lo byte fp8(x), hi byte fp8(x - fp8(x))) and is
   transposed by the DMA XBAR in the prologue (4 group instrs; the XBAR is
   serial + ~28ns/16x128-tile, so it only handles x where nothing contends).
   Matmuls read fp8 planes from the packed layout via stride-2 byte APs.
 - q/attn transposes stay on the PE (bf16 via identity; uint16 Ldweights is
   rejected by the BIR verifier, and the XBAR is too slow mid-pipeline).
 - rope runs on DVE in bf16 (6 fat [128,16,32] ops); scalar pre-copies the
   projection PSUM banks to bf16 (applying the 1/32).
 - vk partial products are single-shot PSUM groups per (tile, pair), DVE-
   accumulated in SBUF (PSUM accumulation groups cannot interleave).
 - phase 2b software-pipelined: num(t) runs ahead of transpose/out_proj(t-1)
   so the PE never waits on the DVE attn normalization.
 - cos/sin tables host-pre-permuted to [128, T*32] bf16 (contiguous load);
   Wq/Wk columns de-interleaved per head host-side (exact through q.k).
"""

import numpy as np

import concourse.bacc as bacc
import concourse.bass as bass
import concourse.mybir as mybir
import concourse.tile as tile
from concourse.masks import make_identity

F32 = mybir.dt.float32
BF16 = mybir.dt.bfloat16
FP8 = mybir.dt.float8e4
U16 = mybir.dt.uint16

D = 1024
H = 16
HD = 64
NPAIR = 8  # head pairs
THETA = 10000.0
EPS = 1e-6
WSCALE = 32.0

B_FULL, S_FULL = 4, 4096
N_CORES = 8
S_LOC_FULL = B_FULL * S_FULL // N_CORES  # 2048

REPLICA_GROUPS = [[0, 1], [2, 3], [4, 5], [6, 7]]

VK_BANK_PAIRS = [(0, 3), (3, 6), (6, 8)]
VKW = 129  # 128 cols of vkT pair + 1 ksum col
NUMW = 65

ACT = mybir.ActivationFunctionType


def build_program(s_loc=S_LOC_FULL, n_cores=N_CORES):
    """Build the SPMD Bass program for one core (all cores run the same code)."""
    T = s_loc // 128
    NG = T // 4  # x transpose groups

    nc = bacc.Bacc("TRN2", target_bir_lowering=False, num_devices=n_cores)

    xp_d = nc.dram_tensor("xp", [s_loc, D], U16, kind="ExternalInput")
    w_d = {
        w: nc.dram_tensor(w, [D, D], FP8, kind="ExternalInput")
        for w in ("wq8", "wqr", "wk8", "wkr", "wv8", "wvr")
    }
    wo_d = nc.dram_tensor("wo", [D, D], BF16, kind="ExternalInput")
    cos_d = nc.dram_tensor("cos_t", [128, T * 32], BF16, kind="ExternalInput")
    sin_d = nc.dram_tensor("sin_t", [128, T * 32], BF16, kind="ExternalInput")
    y_d = nc.dram_tensor("y", [s_loc, D], F32, kind="ExternalOutput")

    def mm(dst, lhsT, rhs, start, stop, perf_mode=None):
        nc.tensor.matmul(
            dst, lhsT=lhsT, rhs=rhs, start=start, stop=stop, perf_mode=perf_mode
        )

    with tile.TileContext(nc) as tc:
        with (
            tc.tile_pool(name="const", bufs=1) as constp,
            tc.tile_pool(name="wpool", bufs=1) as wpool,
            tc.tile_pool(name="vkp", bufs=1) as vkp,
            tc.tile_pool(name="io", bufs=2) as iop,
            tc.tile_pool(name="work", bufs=3) as wk,
            tc.tile_pool(name="psP", bufs=3, space="PSUM") as psP,
            tc.tile_pool(name="psA", bufs=2, space="PSUM") as psA,
            tc.tile_pool(name="dram", bufs=1, space="DRAM") as dramp,
        ):
            # ---- prologue DMAs -------------------------------------------
            # sync: x XBAR transposes (serial on the XBAR anyway)
            # scalar: wk8/wkr (k-proj needs them first), cos/sin
            # gpsimd sw-DGE: remaining weights
            xT_g = [
                wpool.tile([128, 8, 4 * 128], U16, tag=f"xT{g}", name=f"xT{g}")
                for g in range(NG)
            ]
            for g in range(NG):
                nc.sync.dma_start(
                    out=xT_g[g][:],
                    in_=xp_d[g * 512 : (g + 1) * 512, :],
                    transpose=True,
                )

            w_sb = {}

            def load_w(name, eng, dt=FP8):
                dram_t = w_d[name] if name != "wo" else wo_d
                w_sb[name] = wpool.tile([128, 8 * D], dt, tag=name, name="w" + name)
                eng.dma_start(
                    w_sb[name][:].rearrange("p (c n) -> p c n", c=8),
                    dram_t[:].rearrange("(c p) n -> p c n", p=128),
                )

            load_w("wk8", nc.scalar)
            load_w("wkr", nc.scalar)
            cos_all = constp.tile([128, T * 32], BF16)
            nc.scalar.dma_start(cos_all[:], cos_d[:])
            sin_all = constp.tile([128, T * 32], BF16)
            nc.scalar.dma_start(sin_all[:], sin_d[:])
            load_w("wv8", nc.gpsimd)
            load_w("wvr", nc.gpsimd)
            load_w("wq8", nc.gpsimd)
            load_w("wqr", nc.gpsimd)
            load_w("wo", nc.gpsimd, BF16)

            ident = constp.tile([128, 128], F32)
            make_identity(nc, ident[:])
            ident_b = constp.tile([128, 128], BF16)
            nc.vector.tensor_copy(ident_b[:], ident[:])

            cc_in = dramp.tile([128, NPAIR * VKW], F32, tag="cci")
            cc_out = dramp.tile([128, NPAIR * VKW], F32, tag="cco")

            def rope(src_sb, t, dst_ap):
                """rope all 16 heads of one tile: src_sb [128, 1024] bf16
                (de-interleaved: per head 32 evens | 32 odds) -> dst_ap."""
                csb = cos_all[:, t * 32 : (t + 1) * 32]
                ssb = sin_all[:, t * 32 : (t + 1) * 32]
                e3 = src_sb[:].rearrange("p (h d) -> p h d", h=16)[:, :, 0:32]
                o3 = src_sb[:].rearrange("p (h d) -> p h d", h=16)[:, :, 32:64]
                cb = csb.unsqueeze(1).broadcast_to([128, 16, 32])
                sb_ = ssb.unsqueeze(1).broadcast_to([128, 16, 32])
                t1 = wk.tile([128, 512], BF16, tag="ropetmp1")
                t2 = wk.tile([128, 512], BF16, tag="ropetmp2")
                t13 = t1[:].rearrange("p (h d) -> p h d", h=16)
                t23 = t2[:].rearrange("p (h d) -> p h d", h=16)
                d3 = dst_ap.rearrange("p (h d) -> p h d", h=16)
                nc.vector.tensor_mul(t13, e3, cb)
                nc.vector.tensor_mul(t23, o3, sb_)
                nc.vector.tensor_sub(d3[:, :, 0:32], t13, t23)
                nc.vector.tensor_mul(t13, e3, sb_)
                nc.vector.tensor_mul(t23, o3, cb)
                nc.vector.tensor_add(d3[:, :, 32:64], t13, t23)

            def project8(w8, wr, t, dst_cb):
                """fp8 DoubleRow projection of x row-tile t; dst_cb(nb, pk)."""
                g, tt = t // 4, t % 4
                x8 = (
                    xT_g[g][:]
                    .bitcast(FP8)
                    .rearrange("p c (m b) -> p c m b", b=2)[
                        :, :, tt * 128 : (tt + 1) * 128, :
                    ]
                )
                for nb in range(2):
                    pk = psP.tile([128, 512], F32, tag="pp")
                    n_ = 0
                    for off, wt in ((0, w8), (0, wr), (1, w8)):
                        wv = wt[:].rearrange("p (c n) -> p c n", c=8)
                        for cc in range(4):
                            mm(
                                pk[:],
                                x8[:, 2 * cc : 2 * cc + 2, :, off],
                                wv[:, 2 * cc : 2 * cc + 2, nb * 512 : (nb + 1) * 512],
                                start=(n_ == 0),
                                stop=(n_ == 11),
                                perf_mode=mybir.MatmulPerfMode.DoubleRow,
                            )
                            n_ += 1
                    dst_cb(nb, pk)

            def transpose_pe(src_sb, dst_sb, tag):
                """[128, 1024] bf16 -> dst_sb [128, 8, 128] via PE identity."""
                for gg in range(2):
                    pxt = psA.tile([128, 512], BF16, tag="pxt", name="pxt" + tag)
                    for cc in range(4):
                        c = gg * 4 + cc
                        nc.tensor.transpose(
                            pxt[:, cc * 128 : (cc + 1) * 128],
                            src_sb[:, c * 128 : (c + 1) * 128],
                            ident_b[:],
                        )
                    dst = dst_sb[:, gg * 4 : (gg + 1) * 4, :].rearrange(
                        "p c m -> p (c m)"
                    )
                    if gg == 0:
                        nc.scalar.copy(dst, pxt[:])
                    else:
                        nc.vector.tensor_copy(dst, pxt[:])

            # ---------------- phase 1: k, v, vk accumulation ----------------
            phase1 = tc.tile_pool(name="psVK", bufs=2, space="PSUM")
            psVK = phase1.__enter__()
            vkacc = vkp.tile([128, NPAIR * VKW], F32, tag="vkacc", name="vkacc")
            nc.vector.memset(vkacc[:], 0.0)

            for t in range(T):
                # k projection -> bf16 sbuf (scalar, x1/32) -> rope -> relu
                kb = wk.tile([128, D], BF16, tag="ptmp")

                def k_dst(nb, pk, kb=kb):
                    nc.scalar.activation(
                        kb[:, nb * 512 : (nb + 1) * 512],
                        pk[:],
                        ACT.Copy,
                        scale=1.0 / WSCALE,
                    )

                project8(w_sb["wk8"], w_sb["wkr"], t, k_dst)
                kr_sb = wk.tile([128, D], BF16, tag="kr")
                rope(kb, t, kr_sb[:])
                nc.scalar.activation(kr_sb[:], kr_sb[:], ACT.Relu)

                # v projection -> v_sb with ones cols at p*129+128
                v_sb = wk.tile([128, NPAIR * VKW], BF16, tag="v")

                def v_dst(nb, pv, v_sb=v_sb):
                    dst = v_sb[:, nb * 4 * VKW : (nb * 4 + 4) * VKW].rearrange(
                        "p (q c) -> p q c", q=4
                    )[:, :, 0:128]
                    src = pv[:].rearrange("p (q c) -> p q c", q=4)
                    if nb == 0:
                        nc.scalar.activation(dst, src, ACT.Copy, scale=1.0 / WSCALE)
                    else:
                        nc.vector.tensor_scalar_mul(dst, src, 1.0 / WSCALE)

                project8(w_sb["wv8"], w_sb["wvr"], t, v_dst)
                nc.vector.memset(
                    v_sb[:].rearrange("p (q c) -> p q c", q=8)[:, :, 128:129], 1.0
                )

                # vkT partial products: single-shot groups, DVE-accumulated
                for bi, (p0, p1) in enumerate(VK_BANK_PAIRS):
                    pvt = psVK.tile([128, 512], F32, tag="pvt", name=f"pvt{t}_{bi}")
                    for p in range(p0, p1):
                        mm(
                            pvt[:, (p - p0) * VKW : (p - p0 + 1) * VKW],
                            kr_sb[:, p * 128 : (p + 1) * 128],
                            v_sb[:, p * VKW : (p + 1) * VKW],
                            start=True,
                            stop=True,
                        )
                    w_ = (p1 - p0) * VKW
                    nc.vector.tensor_add(
                        vkacc[:, p0 * VKW : p1 * VKW],
                        vkacc[:, p0 * VKW : p1 * VKW],
                        pvt[:, :w_],
                    )

            phase1.__exit__(None, None, None)

            # ---------------- all-reduce vk over sequence-half pairs --------
            nc.gpsimd.dma_start(cc_in[:], vkacc[:])
            nc.gpsimd.collective_compute(
                "AllReduce",
                mybir.AluOpType.add,
                replica_groups=REPLICA_GROUPS,
                ins=[cc_in.opt()],
                outs=[cc_out.opt()],
            )
            vkred = vkp.tile([128, NPAIR * VKW], F32, tag="vkred", name="vkred")
            nc.gpsimd.dma_start(vkred[:], cc_out[:])

            # vkT_sb: head h at partitions (h%2)*64, other half zeroed so num
            # can contract K=128 at row base 0
            vkT_sb = vkp.tile([128, H * NUMW], BF16, tag="vkT")
            nc.vector.memset(vkT_sb[:], 0.0)

            def vk_reorg():
                for h in range(H):
                    p = h // 2
                    if h % 2 == 0:
                        nc.vector.tensor_copy(
                            vkT_sb[0:64, h * NUMW : h * NUMW + 64],
                            vkred[0:64, p * VKW : p * VKW + 64],
                        )
                        nc.vector.tensor_copy(
                            vkT_sb[0:64, h * NUMW + 64 : h * NUMW + 65],
                            vkred[0:64, p * VKW + 128 : p * VKW + 129],
                        )
                    else:
                        nc.vector.tensor_copy(
                            vkT_sb[64:128, h * NUMW : h * NUMW + 65],
                            vkred[64:128, p * VKW + 64 : p * VKW + 129],
                        )

            # ---------------- phase 2a: q proj + rope + PE transpose -------
            qT_all = wpool.tile([128, T, 8, 128], BF16, tag="qT", name="qT_all")
            for t in range(T):
                qb = wk.tile([128, D], BF16, tag="ptmp", name="qb")

                def q_dst(nb, pq, qb=qb):
                    nc.scalar.activation(
                        qb[:, nb * 512 : (nb + 1) * 512],
                        pq[:],
                        ACT.Copy,
                        scale=1.0 / WSCALE,
                    )

                project8(w_sb["wq8"], w_sb["wqr"], t, q_dst)
                qr_sb = wk.tile([128, D], BF16, tag="kr", name="qr_sb")
                rope(qb, t, qr_sb[:])
                nc.scalar.activation(qr_sb[:], qr_sb[:], ACT.Relu)
                transpose_pe(qr_sb, qT_all[:, t], "q")
                if t == 11:
                    vk_reorg()

            # ---------------- phase 2b: num, attn, output ----------------
            def num_attn(t, psN):
                pnum = []
                for bi, (p0, p1) in enumerate(VK_BANK_PAIRS):
                    pn = psN.tile([128, (p1 - p0) * 2 * NUMW], F32, tag="num")
                    pnum.append(pn)
                    for p in range(p0, p1):
                        mm(
                            pn[:, (p - p0) * 2 * NUMW : (p - p0 + 1) * 2 * NUMW],
                            qT_all[:, t, p, :],
                            vkT_sb[:, 2 * p * NUMW : 2 * (p + 1) * NUMW],
                            start=True,
                            stop=True,
                        )

                den = wk.tile([128, H], F32, tag="den")
                for bi, (p0, p1) in enumerate(VK_BANK_PAIRS):
                    nc.vector.tensor_scalar_add(
                        den[:, 2 * p0 : 2 * p1], pnum[bi][:, 64 :: NUMW], EPS
                    )
                rec = wk.tile([128, H], F32, tag="rec")
                nc.vector.reciprocal(rec[:], den[:])

                attn_sb = wk.tile([128, D], BF16, tag="v", name="attn_sb")
                for bi, (p0, p1) in enumerate(VK_BANK_PAIRS):
                    nh = 2 * (p1 - p0)
                    nc.vector.tensor_mul(
                        attn_sb[:, 2 * p0 * 64 : 2 * p1 * 64].rearrange(
                            "p (h e) -> p h e", e=64
                        ),
                        pnum[bi][:, : nh * NUMW].rearrange(
                            "p (h e) -> p h e", e=NUMW
                        )[:, :, 0:64],
                        rec[:, 2 * p0 : 2 * p1]
                        .unsqueeze(2)
                        .broadcast_to([128, nh, 64]),
                    )
                return attn_sb

            def out_proj(t, attn_sb):
                attnT_sb = wk.tile([128, 8, 128], BF16, tag="attnT")
                transpose_pe(attn_sb, attnT_sb[:], "a")
                out_sb = iop.tile([128, D], F32, tag="out")
                wo = w_sb["wo"]
                for nb in range(2):
                    po = psP.tile([128, 512], F32, tag="pp")
                    for c in range(8):
                        mm(
                            po[:],
                            attnT_sb[:, c, :],
                            wo[:, c * D + nb * 512 : c * D + (nb + 1) * 512],
                            start=(c == 0),
                            stop=(c == 7),
                        )
                    if nb == 0:
                        nc.scalar.copy(out_sb[:, 0:512], po[:])
                    else:
                        nc.vector.tensor_copy(out_sb[:, 512:1024], po[:])
                nc.sync.dma_start(y_d[t * 128 : (t + 1) * 128, :], out_sb[:])

            with tc.tile_pool(name="psN", bufs=3, space="PSUM") as psN:
                prev = None
                for t in range(T):
                    at = num_attn(t, psN)
                    if prev is not None:
                        out_proj(t - 1, prev)
                    prev = at
                out_proj(T - 1, prev)

    nc.compile()
    return nc


# ---------------------------------------------------------------------------
# host side
# ---------------------------------------------------------------------------


def _head_perm():
    """De-interleave permutation for Wq/Wk columns (per head: evens then odds)."""
    perm = np.zeros(D, dtype=np.int64)
    for h in range(H):
        for j in range(32):
            perm[h * HD + j] = h * HD + 2 * j
            perm[h * HD + 32 + j] = h * HD + 2 * j + 1
    return perm


def _rope_tables(s_total):
    freqs = 1.0 / (THETA ** (np.arange(0, HD, 2, dtype=np.float64) / HD))
    ang = np.arange(s_total, dtype=np.float64)[:, None] * freqs[None, :]
    return (
        np.cos(ang).astype(np.float32),
        np.sin(ang).astype(np.float32),
    )


def _fp8_split(a):
    import ml_dtypes

    a8 = a.astype(ml_dtypes.float8_e4m3)
    ar = (a - a8.astype(np.float32)).astype(ml_dtypes.float8_e4m3)
    return a8, ar


def make_in_maps(x, Wq, Wk, Wv, Wo, n_cores=N_CORES):
    import ml_dtypes

    bf16 = ml_dtypes.bfloat16
    b, s, d = x.shape
    s_loc = b * s // n_cores
    T = s_loc // 128
    halves = n_cores // b  # sequence splits per batch
    perm = _head_perm()
    wq8, wqr = _fp8_split(np.ascontiguousarray(Wq[:, perm]) * WSCALE)
    wk8, wkr = _fp8_split(np.ascontiguousarray(Wk[:, perm]) * WSCALE)
    wv8, wvr = _fp8_split(np.ascontiguousarray(Wv) * WSCALE)
    Wo16 = np.ascontiguousarray(Wo).astype(bf16)
    cos_full, sin_full = _rope_tables(s)

    x8, xr = _fp8_split(np.ascontiguousarray(x).reshape(-1, d))
    xp_full = (
        x8.view(np.uint8).astype(np.uint16)
        | (xr.view(np.uint8).astype(np.uint16) << 8)
    ).reshape(b, s, d)

    def permute_tab(tab):
        # [s_loc, 32] -> [128, T*32]: out[p, t*32+j] = tab[t*128+p, j]
        return np.ascontiguousarray(
            tab.reshape(T, 128, 32).transpose(1, 0, 2).reshape(128, T * 32)
        ).astype(bf16)

    in_maps = []
    for c in range(n_cores):
        bi, hi = c // halves, c % halves
        r0 = hi * s_loc
        in_maps.append(
            {
                "xp": np.ascontiguousarray(xp_full[bi, r0 : r0 + s_loc]),
                "wq8": wq8,
                "wqr": wqr,
                "wk8": wk8,
                "wkr": wkr,
                "wv8": wv8,
                "wvr": wvr,
                "wo": Wo16,
                "cos_t": permute_tab(cos_full[r0 : r0 + s_loc]),
                "sin_t": permute_tab(sin_full[r0 : r0 + s_loc]),
            }
        )
    return in_maps, s_loc


_CACHED = {}


def kernel(x, Wq, Wk, Wv, Wo):
    from concourse.bass_utils import run_bass_kernel_spmd

    x = np.asarray(x, dtype=np.float32)
    in_maps, s_loc = make_in_maps(
        x,
        np.asarray(Wq, np.float32),
        np.asarray(Wk, np.float32),
        np.asarray(Wv, np.float32),
        np.asarray(Wo, np.float32),
    )
    key = (s_loc, N_CORES)
    if key not in _CACHED:
        _CACHED[key] = build_program(s_loc=s_loc, n_cores=N_CORES)
    nc = _CACHED[key]
    res = run_bass_kernel_spmd(nc, in_maps, list(range(N_CORES)))
    b, s, d = x.shape
    halves = N_CORES // b
    out = np.empty((b, s, d), dtype=np.float32)
    for c in range(N_CORES):
        bi, hi = c // halves, c % halves
        out[bi, hi * s_loc : (hi + 1) * s_loc] = res.results[c]["y"]
    return out
